# revision 26
# baseline (speedup 1.0000x reference)
"""Trainium2 Bass kernel for 3-layer GAT + graph pooling (nn_GATModel).

Edge-stream design (v3): host replicates table rows into a per-edge
slot-aligned fp8 stream between launches (index glue only); device:
  - chunked SWDGE cast-DMA loads (fp8 HBM -> fp16 SBUF), ~96 slots/chunk
  - front-loaded softmax pipeline: one prelu+exp over all logits, per-word
    den/recip/etw (alpha quads, (d,h)-interleaved channel layout)
  - msg = h * alpha: DVE (d-range) + GpSimd (d-range) split, in-place
  - slot-sum: small GpSimd tree + PE identity matmuls at FD=512 (slot pairs)
  - epilogue: transpose-accumulate halves, h_next = out @ [W|Msrc|Mdst]+b
Four launches: L0 (x@W1ext), agg1, agg2, agg3+pool. Host between launches
does index glue only (table assembly + per-edge replication + fp8 encode).
"""

import os
import numpy as np
import ml_dtypes

import concourse.bacc as bacc
import concourse.tile as tile
import concourse.mybir as mybir
from concourse import bass, bass_utils
from contextlib import ExitStack

F16 = mybir.dt.float16
F32 = mybir.dt.float32
F8 = mybir.dt.float8e4
I32 = mybir.dt.int32

N_NODES = 50000
N_EDGES = 800000
N_GRAPHS = 512
HEADS = 4
HDIM = 64
NCORES = 8
NEG_SLOPE = 0.2
LOGIT_M = [6.0, 10.0, 10.0]
ZROW = N_NODES          # table row: h=0, as=+M (safe self for empty slots)
PROW = N_NODES + 1      # table row: h=0, as=-30000 (e == 0 padding)
TROWS = N_NODES + 2
CHUNK_B = 72            # max slots per stream DMA chunk

_EXEC_NS = []


def _trace_on():
    return bool(os.environ.get("GAT_TRACE"))


def _install_profhook():
    import sys, types
    if "antenv.axon_hooks" in sys.modules:
        return True
    try:
        mod = types.ModuleType("antenv.axon_hooks")
        state = {}
        mod.set_axon_ntff_profile_hook = lambda h: state.update(h=h)
        mod.get_axon_ntff_profile_hook = lambda: state.get("h")
        sys.modules["antenv.axon_hooks"] = mod
        sys.path.insert(0, "/root/.axon_site/trn_agent_boot")
        import trn_boot
        mod.set_axon_ntff_profile_hook(
            trn_boot._ntff_profile_via_ctypes("/opt/axon/libaxon_pjrt.so")
        )
        return True
    except Exception:
        sys.modules.pop("antenv.axon_hooks", None)
        return False


# ---------------------------------------------------------------- host prep

def _kg(k):
    return 0


def build_meta(edge_index):
    src = np.asarray(edge_index[0], dtype=np.int64)
    dst = np.asarray(edge_index[1], dtype=np.int64)
    deg = np.bincount(dst, minlength=N_NODES)

    cum = np.cumsum(deg + 1)
    total = cum[-1]
    bounds = [0]
    for c in range(1, NCORES):
        bounds.append(int(np.searchsorted(cum, total * c / NCORES)))
    bounds.append(N_NODES)

    NW = max((bounds[c + 1] - bounds[c] + 127) // 128 for c in range(NCORES))
    maxn = NW * 128

    # edges sorted by dst; slot of edge within its dst segment
    order = np.argsort(dst, kind="stable")
    src_s, dst_s = src[order], dst[order]
    starts = np.searchsorted(dst_s, np.arange(N_NODES))
    slot_e = np.arange(len(src)) - starts[dst_s] + 1  # slot 0 = self

    cores = []
    kT_cores = []
    for c in range(NCORES):
        n0, n1 = bounds[c], bounds[c + 1]
        nodes = np.arange(n0, n1)
        o = np.argsort(deg[nodes], kind="stable")[::-1]
        perm = np.full(maxn, -1, np.int64)
        perm[: n1 - n0] = nodes[o]
        dpad = np.zeros(maxn, np.int64)
        real = perm >= 0
        dpad[real] = deg[perm[real]] + 1
        kc = dpad.reshape(NW, 128).max(1).astype(np.int32)
        cores.append(dict(n0=n0, n1=n1, perm=perm))
        kT_cores.append(np.maximum(kc, 1))
    kT = np.maximum.reduce(kT_cores)
    offs = np.concatenate([[0], np.cumsum(kT)]).astype(np.int64)
    TOT = int(offs[-1])

    # stream DMA chunks: whole words, <= CHUNK_B slots each
    chunks = []
    start = 0
    for w in range(NW):
        if offs[w + 1] - offs[start] > CHUNK_B and w > start:
            chunks.append((start, w))
            start = w
    chunks.append((start, NW))

    for cd in cores:
        perm = cd["perm"]
        posof = np.full(N_NODES, -1, np.int64)
        real = perm >= 0
        posof[perm[real]] = np.nonzero(real)[0]
        idx = np.full((128, TOT), PROW, np.int32)
        # self slots
        pos = posof[perm[real]]
        w, p = pos // 128, pos % 128
        idx[p, offs[w]] = perm[real]
        # empty partitions: self -> ZROW
        epos = np.nonzero(~real)[0]
        idx[epos % 128, offs[epos // 128]] = ZROW
        # edges owned by this core
        sel = (dst_s >= cd["n0"]) & (dst_s < cd["n1"])
        pos = posof[dst_s[sel]]
        w, p = pos // 128, pos % 128
        idx[p, offs[w] + slot_e[sel]] = src_s[sel].astype(np.int32)
        cd["idx"] = idx

    return dict(NW=NW, kT=kT, offs=offs, TOT=TOT, cores=cores, chunks=chunks)


def build_pool(meta, batch):
    batch = np.asarray(batch, dtype=np.int64)
    NW = meta["NW"]
    for cd in meta["cores"]:
        perm = cd["perm"]
        gbase = int(batch[cd["n0"]])
        gspan = int(batch[cd["n1"] - 1]) - gbase + 1
        assert gspan <= 128
        # onehot layout [128, NW*128]: oh[p, w*128+g]
        oh = np.zeros((128, NW, 128), np.float16)
        real = perm >= 0
        pos = np.nonzero(real)[0]
        oh[pos % 128, pos // 128, batch[perm[real]] - gbase] = 1.0
        cd["pool_onehot"] = oh.reshape(128, NW * 128)
        cd["gbase"] = gbase
    meta["gcounts"] = np.bincount(batch, minlength=N_GRAPHS).astype(np.float64)


def make_wext(W, a_s, a_d):
    """[Fin, 256] params -> [Fin, 264] f32 with as/ad columns fused."""
    W = np.asarray(W, np.float64)
    a_s = np.asarray(a_s, np.float64)
    a_d = np.asarray(a_d, np.float64)
    W3 = W.reshape(W.shape[0], HEADS, HDIM)
    ms = np.einsum("chd,hd->ch", W3, a_s)
    md = np.einsum("chd,hd->ch", W3, a_d)
    return np.concatenate([W, ms, md], axis=1).astype(np.float32)


def make_bext(b, a_s, a_d):
    b = np.asarray(b, np.float64)
    bs = np.einsum("hd,hd->h", b.reshape(HEADS, HDIM), np.asarray(a_s, np.float64))
    bd = np.einsum("hd,hd->h", b.reshape(HEADS, HDIM), np.asarray(a_d, np.float64))
    return np.concatenate([b, bs, bd]).astype(np.float32)


# ---------------------------------------------------------------- device util

def _ap(t_ap, off_elems, dims):
    return bass.AP(t_ap.tensor, t_ap.offset + off_elems, dims)


# ---------------------------------------------------------------- programs

def build_l0(meta):
    NW = meta["NW"]
    nc = bacc.Bacc("TRN2", target_bir_lowering=False, debug=False, num_devices=NCORES)
    xT = nc.dram_tensor("xT", [128, NW * 128], F16, kind="ExternalInput").ap()
    Wext = nc.dram_tensor("Wext", [128, 264], F16, kind="ExternalInput").ap()
    bext = nc.dram_tensor("bext", [1, 264], F16, kind="ExternalInput").ap()
    ones1 = nc.dram_tensor("ones1", [1, 128], F16, kind="ExternalInput").ap()
    # partition-major hout: [128, NW*264]
    hout = nc.dram_tensor("hout", [128, NW * 264], F16, kind="ExternalOutput").ap()

    WGRP = 13  # words per output DMA group

    with ExitStack() as ctx:
        tc = ctx.enter_context(tile.TileContext(nc))
        cpool = ctx.enter_context(tc.tile_pool(name="c", bufs=1))
        pspool = ctx.enter_context(tc.tile_pool(name="ps", bufs=4, space="PSUM"))
        W_s = cpool.tile([128, 264], F16)
        nc.sync.dma_start(W_s[:], Wext[:])
        b_s = cpool.tile([1, 264], F16)
        nc.sync.dma_start(b_s[:], bext[:])
        o_s = cpool.tile([1, 128], F16)
        nc.sync.dma_start(o_s[:], ones1[:])
        xT_s = cpool.tile([128, NW * 128], F16)
        nc.sync.dma_start(xT_s[:], xT[:])
        stage = cpool.tile([128, NW * 264], F16)
        for w in range(NW):
            hp = pspool.tile([128, 264], F32, tag="hp")
            nc.tensor.matmul(hp[:], lhsT=xT_s[:, w * 128:(w + 1) * 128], rhs=W_s[:],
                             start=True, stop=False)
            nc.tensor.matmul(hp[:], lhsT=o_s[:], rhs=b_s[:], start=False, stop=True)
            nc.scalar.copy(stage[:, w * 264:(w + 1) * 264], hp[:])
            if (w + 1) % WGRP == 0 or w == NW - 1:
                g0 = (w // WGRP) * WGRP
                nc.sync.dma_start(hout[:, g0 * 264:(w + 1) * 264],
                                  stage[:, g0 * 264:(w + 1) * 264])
    nc.compile()
    return nc


def build_agg(meta, last):
    NW, kT, offs, TOT = meta["NW"], meta["kT"], meta["offs"], meta["TOT"]
    chunks = meta["chunks"]
    nc = bacc.Bacc("TRN2", target_bir_lowering=False, debug=False, num_devices=NCORES)
    stream = nc.dram_tensor("stream", [128, TOT * 256], F8, kind="ExternalInput").ap()
    asst = nc.dram_tensor("asst", [128, TOT * 4], F16, kind="ExternalInput").ap()
    adin = nc.dram_tensor("adin", [128, NW * 4], F16, kind="ExternalInput").ap()
    mshift = nc.dram_tensor("mshift", [128, 1], F32, kind="ExternalInput").ap()
    ident16 = nc.dram_tensor("ident16", [128, 128], F16, kind="ExternalInput").ap()
    if not last:
        Wext = nc.dram_tensor("Wext", [128, 2 * 264], F16, kind="ExternalInput").ap()
        bext = nc.dram_tensor("bext", [1, 264], F16, kind="ExternalInput").ap()
        ones1 = nc.dram_tensor("ones1", [1, 128], F16, kind="ExternalInput").ap()
        hout = nc.dram_tensor("hout", [128, NW * 264], F16, kind="ExternalOutput").ap()
    else:
        onehot = nc.dram_tensor("onehot", [128, NW * 128], F16, kind="ExternalInput").ap()
        fcw = nc.dram_tensor("fcw", [128, 256], F32, kind="ExternalInput").ap()
        pout = nc.dram_tensor("pout", [128, 1], F32, kind="ExternalOutput").ap()

    SMAX = max(int(offs[c1] - offs[c0]) for c0, c1 in chunks)
    WCH = max(c1 - c0 for c0, c1 in chunks)
    NCH = len(chunks)
    # as_s loaded in two pieces; the tail piece (processed first) loads first
    asplit = chunks[max(NCH - 4, 1)][0]

    with ExitStack() as ctx:
        tc = ctx.enter_context(tile.TileContext(nc))
        cpool = ctx.enter_context(tc.tile_pool(name="c", bufs=1))
        gpool = ctx.enter_context(tc.tile_pool(name="g", bufs=3))
        spool = ctx.enter_context(tc.tile_pool(name="s", bufs=4))
        ofpool = ctx.enter_context(tc.tile_pool(name="of", bufs=2))
        hopool = ctx.enter_context(tc.tile_pool(name="ho", bufs=2))
        pspool = ctx.enter_context(tc.tile_pool(name="ps", bufs=3, space="PSUM"))
        ptpool = ctx.enter_context(tc.tile_pool(name="pt", bufs=2, space="PSUM"))
        pxpool = ctx.enter_context(tc.tile_pool(name="px", bufs=2, space="PSUM"))
        pppool = ctx.enter_context(tc.tile_pool(name="pp", bufs=1, space="PSUM"))

        # small tiles first on the sync queue; big as_s pieces last
        ad_s = cpool.tile([128, NW * 4], F16)
        nc.sync.dma_start(ad_s[:], adin[:])
        msh_s = cpool.tile([128, 1], F32)
        nc.sync.dma_start(msh_s[:], mshift[:])
        id16 = cpool.tile([128, 128], F16)
        nc.sync.dma_start(id16[:], ident16[:])
        alslope = cpool.tile([128, 1], F32)
        nc.vector.memset(alslope[:], NEG_SLOPE)
        if not last:
            W_s = cpool.tile([128, 2 * 264], F16)
            nc.sync.dma_start(W_s[:], Wext[:])
            b_s = cpool.tile([1, 264], F16)
            nc.sync.dma_start(b_s[:], bext[:])
            o_s = cpool.tile([1, 128], F16)
            nc.sync.dma_start(o_s[:], ones1[:])
        else:
            fcw_s = cpool.tile([128, 256], F32)
            nc.sync.dma_start(fcw_s[:], fcw[:])
            pp = pppool.tile([128, 256], F32)
        aoff = int(offs[asplit]) * 4
        as_s = cpool.tile([128, TOT * 4], F16)
        nc.sync.dma_start(as_s[:, aoff:], asst[:, aoff:])
        nc.sync.dma_start(as_s[:, :aoff], asst[:, :aoff])
        if last:
            oh_s = cpool.tile([128, NW * 128], F16)
            nc.sync.dma_start(oh_s[:], onehot[:])

        # lg -> (prelu) lr -> (exp) back into lg: lg doubles as e_all
        lg = cpool.tile([128, TOT * 4], F16)
        lr = cpool.tile([128, TOT * 4], F16)
        e_all = lg
        etw = cpool.tile([128, TOT * 8], F16)
        den = cpool.tile([128, NW * 4], F32)
        rde = cpool.tile([128, NW * 4], F32)

        def softmax_chunk(ci):
            c0, c1 = chunks[ci]
            o4c, o4e = int(offs[c0]) * 4, int(offs[c1]) * 4
            for w in range(c0, c1):
                k = int(kT[w])
                o4 = int(offs[w]) * 4
                ad_b = _ap(ad_s[:], w * 4, [list(ad_s[:].ap[0]), [0, k], [1, 4]])
                as_v = _ap(as_s[:], o4, [list(as_s[:].ap[0]), [4, k], [1, 4]])
                nc.vector.tensor_tensor(
                    out=_ap(lg[:], o4, [list(lg[:].ap[0]), [4, k], [1, 4]]),
                    in0=as_v, in1=ad_b, op=mybir.AluOpType.add)
            nc.scalar.activation(lr[:, o4c:o4e], lg[:, o4c:o4e],
                                 mybir.ActivationFunctionType.Prelu,
                                 alpha=alslope[:])
            nc.scalar.activation(e_all[:, o4c:o4e], lr[:, o4c:o4e],
                                 mybir.ActivationFunctionType.Exp,
                                 bias=msh_s[:], scale=1.0)
            for w in range(c0, c1):
                k = int(kT[w])
                o4 = int(offs[w]) * 4
                nc.vector.reduce_sum(
                    den[:, w * 4:(w + 1) * 4],
                    _ap(e_all[:], o4, [list(e_all[:].ap[0]), [1, 4], [4, k]]),
                    axis=mybir.AxisListType.X)
            nc.vector.reciprocal(rde[:, c0 * 4:c1 * 4], den[:, c0 * 4:c1 * 4])
            for w in range(c0, c1):
                k = int(kT[w])
                o4 = int(offs[w]) * 4
                nc.vector.tensor_tensor(
                    out=_ap(etw[:], int(offs[w]) * 8,
                            [list(etw[:].ap[0]), [8, k], [2, 4], [1, 2]]),
                    in0=_ap(e_all[:], o4,
                            [list(e_all[:].ap[0]), [4, k], [1, 4], [0, 2]]),
                    in1=_ap(rde[:], w * 4,
                            [list(rde[:].ap[0]), [0, k], [1, 4], [0, 2]]),
                    op=mybir.AluOpType.mult)

        gtiles = {}

        def issue_chunk(ci):
            c0, c1 = chunks[ci]
            S = int(offs[c1] - offs[c0])
            g = gpool.tile([128, SMAX * 256], F16, tag="g")
            nc.gpsimd.dma_start(
                g[:, :S * 256],
                stream[:, int(offs[c0]) * 256:int(offs[c1]) * 256])
            gtiles[ci] = g

        # process chunks in reverse (last chunks have many small words;
        # ending on a few-word chunk shortens the epilogue tail)
        order = list(range(NCH))[::-1]
        issue_chunk(order[0])
        if NCH > 1:
            issue_chunk(order[1])
        softmax_chunk(order[0])

        pend = None      # deferred epilogue: (kind, args)
        hos_pend = None  # (hos_tile, c0, c1) awaiting last-word epilogue + DMA

        def emit_epilogue(pw):
            w, ofs_t, ob, hos_t, hb = pw
            outT = spool.tile([128, 256], F16, tag="outT")
            pt = ptpool.tile([128, 256], F16, tag="pt")
            for q in range(2):
                nc.tensor.matmul(pt[:, q * 128:(q + 1) * 128],
                                 lhsT=ofs_t[:, ob + q * 128:ob + (q + 1) * 128],
                                 rhs=id16[:], is_transpose=True,
                                 start=True, stop=True,
                                 skip_group_check=True)
            nc.scalar.copy(outT[:], pt[:])
            hp = pxpool.tile([128, 264], F32, tag="hp")
            for q in range(2):
                nc.tensor.matmul(hp[:], lhsT=outT[:, q * 128:(q + 1) * 128],
                                 rhs=W_s[:, q * 264:(q + 1) * 264],
                                 start=(q == 0), stop=False)
            nc.tensor.matmul(hp[:], lhsT=o_s[:], rhs=b_s[:],
                             start=False, stop=True)
            nc.scalar.copy(hos_t[:, hb * 264:(hb + 1) * 264], hp[:])

        for oi, ci in enumerate(order):
            c0, c1 = chunks[ci]
            if oi + 2 < NCH:
                issue_chunk(order[oi + 2])
            if oi + 1 < NCH:
                softmax_chunk(order[oi + 1])
            g = gtiles.pop(ci)
            nw_c = c1 - c0
            # msg = h * alpha for the whole chunk (one DVE op, 2x mode)
            S = int(offs[c1] - offs[c0])
            ge = _ap(g[:], 0, [list(g[:].ap[0]), [64, S * 4], [2, 32], [1, 2]])
            ee = _ap(etw[:], int(offs[c0]) * 8,
                     [list(etw[:].ap[0]), [2, S * 4], [0, 32], [1, 2]])
            nc.vector.tensor_tensor(out=ge, in0=ge, in1=ee, op=mybir.AluOpType.mult)
            of = ofpool.tile([128, WCH * 512], F16, tag="of")
            if not last:
                hos = hopool.tile([128, WCH * 264], F16, tag="hos")
                ofs = ofpool.tile([128, WCH * 256], F16, tag="ofs")
            for w in range(c0, c1):
                k = int(kT[w])
                gb = (int(offs[w]) - int(offs[c0])) * 256  # elem offset in g
                # slot-sum: FD=512 pairs into psum halves, odd tail into half A
                ps = pspool.tile([128, 512], F32, tag="ps")
                np2 = k // 2
                for t in range(np2):
                    nc.tensor.matmul(
                        ps[:], lhsT=id16[:],
                        rhs=_ap(g[:], gb + 2 * t * 256,
                                [list(g[:].ap[0]), [1, 512]]),
                        start=(t == 0), stop=(t == np2 - 1 and k % 2 == 0))
                if k % 2 == 1:
                    nc.tensor.matmul(
                        ps[:, 0:256], lhsT=id16[:],
                        rhs=_ap(g[:], gb + (k - 1) * 256,
                                [list(g[:].ap[0]), [1, 256]]),
                        start=False, stop=True, skip_group_check=True)
                ob = (w - c0) * 512
                nc.scalar.copy(of[:, ob:ob + 512], ps[:])
                if not last:
                    sb = (w - c0) * 256
                    nc.vector.tensor_tensor(
                        out=ofs[:, sb:sb + 256], in0=of[:, ob:ob + 256],
                        in1=of[:, ob + 256:ob + 512], op=mybir.AluOpType.add)
                    if pend is not None:
                        emit_epilogue(pend)
                    pend = (w, ofs, sb, hos, w - c0)
                else:
                    nc.tensor.matmul(pp[:], lhsT=oh_s[:, w * 128:(w + 1) * 128],
                                     rhs=of[:, ob:ob + 256],
                                     start=(oi == 0 and w == c0), stop=False,
                                     skip_group_check=True)
                    last_word = (oi == NCH - 1 and w == c1 - 1)
                    nc.tensor.matmul(pp[:], lhsT=oh_s[:, w * 128:(w + 1) * 128],
                                     rhs=of[:, ob + 256:ob + 512],
                                     start=False, stop=last_word,
                                     skip_group_check=True)
            if not last:
                if hos_pend is not None:
                    # flush previous chunk: its last word is in pend? no --
                    # pend now belongs to this chunk; previous chunk fully
                    # epilogued except its last word, which was emitted as the
                    # first pend flush above. DMA it out now.
                    ph, pc0, pc1 = hos_pend
                    nc.sync.dma_start(hout[:, pc0 * 264:pc1 * 264],
                                      ph[:, :(pc1 - pc0) * 264])
                hos_pend = (hos, c0, c1)
        if not last:
            if pend is not None:
                emit_epilogue(pend)
            if hos_pend is not None:
                ph, pc0, pc1 = hos_pend
                nc.sync.dma_start(hout[:, pc0 * 264:pc1 * 264],
                                  ph[:, :(pc1 - pc0) * 264])
        if last:
            fm = spool.tile([128, 256], F32, tag="fm")
            nc.vector.tensor_tensor(out=fm[:], in0=pp[:], in1=fcw_s[:],
                                    op=mybir.AluOpType.mult)
            pv = spool.tile([128, 1], F32, tag="pv")
            nc.vector.reduce_sum(pv[:], fm[:], axis=mybir.AxisListType.X)
            nc.sync.dma_start(pout[:], pv[:])
    nc.compile()
    return nc


# ---------------------------------------------------------------- run

def _run(nc, in_maps):
    trace = _trace_on() and _install_profhook()
    res = bass_utils.run_bass_kernel_spmd(
        nc, in_maps=in_maps, core_ids=list(range(NCORES)), trace=trace
    )
    if _trace_on():
        _EXEC_NS.append(res.exec_time_ns)
    return res


def kernel(x, edge_index, batch, W1, a_src1, a_dst1, b1, W2, a_src2, a_dst2, b2,
           W3, a_src3, a_dst3, b3, fc_W, fc_b):
    _EXEC_NS.clear()
    x = np.asarray(x, np.float32)
    edge_index = np.asarray(edge_index)
    batch = np.asarray(batch)
    meta = build_meta(edge_index)
    build_pool(meta, batch)
    NW, TOT = meta["NW"], meta["TOT"]
    id16 = np.eye(128, dtype=np.float16)
    ones1 = np.ones((1, 128), np.float16)

    wext1 = make_wext(W1, a_src1, a_dst1).astype(np.float16)
    bext1 = make_bext(b1, a_src1, a_dst1).astype(np.float16).reshape(1, 264)
    nc0 = build_l0(meta)
    in0 = []
    for cd in meta["cores"]:
        xp = np.zeros((NW * 128, 128), np.float16)
        real = cd["perm"] >= 0
        xp[real] = x[cd["perm"][real]].astype(np.float16)
        in0.append({"xT": np.ascontiguousarray(xp.T), "Wext": wext1, "bext": bext1,
                    "ones1": ones1})
    r0 = _run(nc0, in0)
    # hout layout [128, NW*264] -> per-node rows [NW*128, 264]
    houts = [r0.results[c]["hout"].reshape(128, NW, 264)
             .transpose(1, 0, 2).reshape(NW * 128, 264)
             for c in range(NCORES)]

    nc_mid = build_agg(meta, last=False)
    nc_last = build_agg(meta, last=True)

    wexts = [make_wext(W2, a_src2, a_dst2).astype(np.float16),
             make_wext(W3, a_src3, a_dst3).astype(np.float16), None]
    bexts = [make_bext(b2, a_src2, a_dst2).astype(np.float16).reshape(1, 264),
             make_bext(b3, a_src3, a_dst3).astype(np.float16).reshape(1, 264), None]

    for li in range(3):
        last = li == 2
        # assemble global tables from per-core houts
        tab_h = np.zeros((TROWS, 256), np.float32)
        tab_as = np.zeros((TROWS, 4), np.float16)
        for cd, h in zip(meta["cores"], houts):
            real = cd["perm"] >= 0
            tab_h[cd["perm"][real]] = h[real, 0:256].astype(np.float32)
            tab_as[cd["perm"][real]] = h[real, 256:260]
        tab_as[ZROW] = np.float16(LOGIT_M[li])
        tab_as[PROW] = np.float16(-30000.0)
        tab8 = np.clip(tab_h, -240, 240).astype(ml_dtypes.float8_e4m3fn)

        ims = []
        for c, cd in enumerate(meta["cores"]):
            stream = tab8[cd["idx"]].reshape(128, TOT * 256)
            asst = tab_as[cd["idx"]].reshape(128, TOT * 4)
            adin = np.ascontiguousarray(
                houts[c].reshape(NW, 128, 264)[:, :, 260:264]
                .transpose(1, 0, 2).reshape(128, NW * 4)).astype(np.float16)
            im = {"stream": stream, "asst": asst, "adin": adin,
                  "mshift": np.full((128, 1), -LOGIT_M[li], np.float32),
                  "ident16": id16}
            if not last:
                W2c = wexts[li]  # [256, 264]
                im["Wext"] = np.ascontiguousarray(
                    np.concatenate([W2c[0:128, :], W2c[128:256, :]], axis=1))
                im["bext"] = bexts[li]
                im["ones1"] = ones1
            else:
                im["onehot"] = cd["pool_onehot"]
                im["fcw"] = np.tile(np.asarray(fc_W, np.float32).reshape(1, 256), (128, 1))
            ims.append(im)
        rr = _run(nc_mid if not last else nc_last, ims)
        if not last:
            houts = [rr.results[c]["hout"].reshape(128, NW, 264)
                     .transpose(1, 0, 2).reshape(NW * 128, 264)
                     for c in range(NCORES)]
        else:
            outv = np.zeros(N_GRAPHS, np.float64)
            for c, cd in enumerate(meta["cores"]):
                pv = rr.results[c]["pout"].reshape(128)
                gb = cd["gbase"]
                hi = min(128, N_GRAPHS - gb)
                outv[gb:gb + hi] += pv[:hi]
            bias_fc = float(np.asarray(b3, np.float64) @ np.asarray(fc_W, np.float64).reshape(-1))
            outv += meta["gcounts"] * bias_fc
            outv += float(np.asarray(fc_b, np.float64).reshape(()))
    return outv.reshape(N_GRAPHS, 1).astype(np.float32)


# revision 27
# speedup vs baseline: 1.0144x; 1.0144x over previous
"""Trainium2 Bass kernel for 3-layer GAT + graph pooling (nn_GATModel).

Edge-stream design (v3): host replicates table rows into a per-edge
slot-aligned fp8 stream between launches (index glue only); device:
  - chunked SWDGE cast-DMA loads (fp8 HBM -> fp16 SBUF), ~96 slots/chunk
  - front-loaded softmax pipeline: one prelu+exp over all logits, per-word
    den/recip/etw (alpha quads, (d,h)-interleaved channel layout)
  - msg = h * alpha: DVE (d-range) + GpSimd (d-range) split, in-place
  - slot-sum: small GpSimd tree + PE identity matmuls at FD=512 (slot pairs)
  - epilogue: transpose-accumulate halves, h_next = out @ [W|Msrc|Mdst]+b
Four launches: L0 (x@W1ext), agg1, agg2, agg3+pool. Host between launches
does index glue only (table assembly + per-edge replication + fp8 encode).
"""

import os
import numpy as np
import ml_dtypes

import concourse.bacc as bacc
import concourse.tile as tile
import concourse.mybir as mybir
from concourse import bass, bass_utils
from contextlib import ExitStack

F16 = mybir.dt.float16
F32 = mybir.dt.float32
F8 = mybir.dt.float8e4
I32 = mybir.dt.int32

N_NODES = 50000
N_EDGES = 800000
N_GRAPHS = 512
HEADS = 4
HDIM = 64
NCORES = 8
NEG_SLOPE = 0.2
LOGIT_M = [6.0, 10.0, 10.0]
ZROW = N_NODES          # table row: h=0, as=+M (safe self for empty slots)
PROW = N_NODES + 1      # table row: h=0, as=-30000 (e == 0 padding)
TROWS = N_NODES + 2
CHUNK_B = 72            # max slots per stream DMA chunk

_EXEC_NS = []


def _trace_on():
    return bool(os.environ.get("GAT_TRACE"))


def _install_profhook():
    import sys, types
    if "antenv.axon_hooks" in sys.modules:
        return True
    try:
        mod = types.ModuleType("antenv.axon_hooks")
        state = {}
        mod.set_axon_ntff_profile_hook = lambda h: state.update(h=h)
        mod.get_axon_ntff_profile_hook = lambda: state.get("h")
        sys.modules["antenv.axon_hooks"] = mod
        sys.path.insert(0, "/root/.axon_site/trn_agent_boot")
        import trn_boot
        mod.set_axon_ntff_profile_hook(
            trn_boot._ntff_profile_via_ctypes("/opt/axon/libaxon_pjrt.so")
        )
        return True
    except Exception:
        sys.modules.pop("antenv.axon_hooks", None)
        return False


# ---------------------------------------------------------------- host prep

def _kg(k):
    return 0


def build_meta(edge_index):
    src = np.asarray(edge_index[0], dtype=np.int64)
    dst = np.asarray(edge_index[1], dtype=np.int64)
    deg = np.bincount(dst, minlength=N_NODES)

    cum = np.cumsum(deg + 1)
    total = cum[-1]
    bounds = [0]
    for c in range(1, NCORES):
        bounds.append(int(np.searchsorted(cum, total * c / NCORES)))
    bounds.append(N_NODES)

    NW = max((bounds[c + 1] - bounds[c] + 127) // 128 for c in range(NCORES))
    maxn = NW * 128

    # edges sorted by dst; slot of edge within its dst segment
    order = np.argsort(dst, kind="stable")
    src_s, dst_s = src[order], dst[order]
    starts = np.searchsorted(dst_s, np.arange(N_NODES))
    slot_e = np.arange(len(src)) - starts[dst_s] + 1  # slot 0 = self

    cores = []
    kT_cores = []
    for c in range(NCORES):
        n0, n1 = bounds[c], bounds[c + 1]
        nodes = np.arange(n0, n1)
        o = np.argsort(deg[nodes], kind="stable")[::-1]
        perm = np.full(maxn, -1, np.int64)
        perm[: n1 - n0] = nodes[o]
        dpad = np.zeros(maxn, np.int64)
        real = perm >= 0
        dpad[real] = deg[perm[real]] + 1
        kc = dpad.reshape(NW, 128).max(1).astype(np.int32)
        cores.append(dict(n0=n0, n1=n1, perm=perm))
        kT_cores.append(np.maximum(kc, 1))
    kT = np.maximum.reduce(kT_cores)
    offs = np.concatenate([[0], np.cumsum(kT)]).astype(np.int64)
    TOT = int(offs[-1])

    # stream DMA chunks: whole words, <= CHUNK_B slots each
    chunks = []
    start = 0
    for w in range(NW):
        if offs[w + 1] - offs[start] > CHUNK_B and w > start:
            chunks.append((start, w))
            start = w
    chunks.append((start, NW))

    for cd in cores:
        perm = cd["perm"]
        posof = np.full(N_NODES, -1, np.int64)
        real = perm >= 0
        posof[perm[real]] = np.nonzero(real)[0]
        idx = np.full((128, TOT), PROW, np.int32)
        # self slots
        pos = posof[perm[real]]
        w, p = pos // 128, pos % 128
        idx[p, offs[w]] = perm[real]
        # empty partitions: self -> ZROW
        epos = np.nonzero(~real)[0]
        idx[epos % 128, offs[epos // 128]] = ZROW
        # edges owned by this core
        sel = (dst_s >= cd["n0"]) & (dst_s < cd["n1"])
        pos = posof[dst_s[sel]]
        w, p = pos // 128, pos % 128
        idx[p, offs[w] + slot_e[sel]] = src_s[sel].astype(np.int32)
        cd["idx"] = idx

    return dict(NW=NW, kT=kT, offs=offs, TOT=TOT, cores=cores, chunks=chunks)


def build_pool(meta, batch):
    batch = np.asarray(batch, dtype=np.int64)
    NW = meta["NW"]
    for cd in meta["cores"]:
        perm = cd["perm"]
        gbase = int(batch[cd["n0"]])
        gspan = int(batch[cd["n1"] - 1]) - gbase + 1
        assert gspan <= 128
        # onehot layout [128, NW*128]: oh[p, w*128+g]
        oh = np.zeros((128, NW, 128), np.float16)
        real = perm >= 0
        pos = np.nonzero(real)[0]
        oh[pos % 128, pos // 128, batch[perm[real]] - gbase] = 1.0
        cd["pool_onehot"] = oh.reshape(128, NW * 128)
        cd["gbase"] = gbase
    meta["gcounts"] = np.bincount(batch, minlength=N_GRAPHS).astype(np.float64)


def make_wext(W, a_s, a_d):
    """[Fin, 256] params -> [Fin, 264] f32 with as/ad columns fused."""
    W = np.asarray(W, np.float64)
    a_s = np.asarray(a_s, np.float64)
    a_d = np.asarray(a_d, np.float64)
    W3 = W.reshape(W.shape[0], HEADS, HDIM)
    ms = np.einsum("chd,hd->ch", W3, a_s)
    md = np.einsum("chd,hd->ch", W3, a_d)
    return np.concatenate([W, ms, md], axis=1).astype(np.float32)


def make_bext(b, a_s, a_d):
    b = np.asarray(b, np.float64)
    bs = np.einsum("hd,hd->h", b.reshape(HEADS, HDIM), np.asarray(a_s, np.float64))
    bd = np.einsum("hd,hd->h", b.reshape(HEADS, HDIM), np.asarray(a_d, np.float64))
    return np.concatenate([b, bs, bd]).astype(np.float32)


# ---------------------------------------------------------------- device util

def _ap(t_ap, off_elems, dims):
    return bass.AP(t_ap.tensor, t_ap.offset + off_elems, dims)


# ---------------------------------------------------------------- programs

def build_l0(meta):
    NW = meta["NW"]
    nc = bacc.Bacc("TRN2", target_bir_lowering=False, debug=False, num_devices=NCORES)
    xT = nc.dram_tensor("xT", [128, NW * 128], F16, kind="ExternalInput").ap()
    Wext = nc.dram_tensor("Wext", [128, 264], F16, kind="ExternalInput").ap()
    bext = nc.dram_tensor("bext", [1, 264], F16, kind="ExternalInput").ap()
    ones1 = nc.dram_tensor("ones1", [1, 128], F16, kind="ExternalInput").ap()
    # partition-major hout: [128, NW*264]
    hout = nc.dram_tensor("hout", [128, NW * 264], F16, kind="ExternalOutput").ap()

    WGRP = 13  # words per output DMA group

    with ExitStack() as ctx:
        tc = ctx.enter_context(tile.TileContext(nc))
        cpool = ctx.enter_context(tc.tile_pool(name="c", bufs=1))
        pspool = ctx.enter_context(tc.tile_pool(name="ps", bufs=4, space="PSUM"))
        W_s = cpool.tile([128, 264], F16)
        nc.sync.dma_start(W_s[:], Wext[:])
        b_s = cpool.tile([1, 264], F16)
        nc.sync.dma_start(b_s[:], bext[:])
        o_s = cpool.tile([1, 128], F16)
        nc.sync.dma_start(o_s[:], ones1[:])
        xT_s = cpool.tile([128, NW * 128], F16)
        nc.sync.dma_start(xT_s[:], xT[:])
        stage = cpool.tile([128, NW * 264], F16)
        for w in range(NW):
            hp = pspool.tile([128, 264], F32, tag="hp")
            nc.tensor.matmul(hp[:], lhsT=xT_s[:, w * 128:(w + 1) * 128], rhs=W_s[:],
                             start=True, stop=False)
            nc.tensor.matmul(hp[:], lhsT=o_s[:], rhs=b_s[:], start=False, stop=True)
            nc.scalar.copy(stage[:, w * 264:(w + 1) * 264], hp[:])
            if (w + 1) % WGRP == 0 or w == NW - 1:
                g0 = (w // WGRP) * WGRP
                nc.sync.dma_start(hout[:, g0 * 264:(w + 1) * 264],
                                  stage[:, g0 * 264:(w + 1) * 264])
    nc.compile()
    return nc


def build_agg(meta, last):
    NW, kT, offs, TOT = meta["NW"], meta["kT"], meta["offs"], meta["TOT"]
    chunks = meta["chunks"]
    nc = bacc.Bacc("TRN2", target_bir_lowering=False, debug=False, num_devices=NCORES)
    stream = nc.dram_tensor("stream", [128, TOT * 256], F8, kind="ExternalInput").ap()
    asst = nc.dram_tensor("asst", [128, TOT * 4], F16, kind="ExternalInput").ap()
    adin = nc.dram_tensor("adin", [128, NW * 4], F16, kind="ExternalInput").ap()
    mshift = nc.dram_tensor("mshift", [128, 1], F32, kind="ExternalInput").ap()
    ident16 = nc.dram_tensor("ident16", [128, 128], F16, kind="ExternalInput").ap()
    if not last:
        Wext = nc.dram_tensor("Wext", [128, 2 * 264], F16, kind="ExternalInput").ap()
        bext = nc.dram_tensor("bext", [1, 264], F16, kind="ExternalInput").ap()
        ones1 = nc.dram_tensor("ones1", [1, 128], F16, kind="ExternalInput").ap()
        hout = nc.dram_tensor("hout", [128, NW * 264], F16, kind="ExternalOutput").ap()
    else:
        onehot = nc.dram_tensor("onehot", [128, NW * 128], F16, kind="ExternalInput").ap()
        fcw = nc.dram_tensor("fcw", [128, 256], F32, kind="ExternalInput").ap()
        pout = nc.dram_tensor("pout", [128, 1], F32, kind="ExternalOutput").ap()

    SMAX = max(int(offs[c1] - offs[c0]) for c0, c1 in chunks)
    WCH = max(c1 - c0 for c0, c1 in chunks)
    NCH = len(chunks)
    # as_s loaded in two pieces; the tail piece (processed first) loads first
    asplit = chunks[max(NCH - 4, 1)][0]

    with ExitStack() as ctx:
        tc = ctx.enter_context(tile.TileContext(nc))
        cpool = ctx.enter_context(tc.tile_pool(name="c", bufs=1))
        gpool = ctx.enter_context(tc.tile_pool(name="g", bufs=3))
        spool = ctx.enter_context(tc.tile_pool(name="s", bufs=4))
        ofpool = ctx.enter_context(tc.tile_pool(name="of", bufs=2))
        hopool = ctx.enter_context(tc.tile_pool(name="ho", bufs=2))
        pspool = ctx.enter_context(tc.tile_pool(name="ps", bufs=3, space="PSUM"))
        ptpool = ctx.enter_context(tc.tile_pool(name="pt", bufs=2, space="PSUM"))
        pxpool = ctx.enter_context(tc.tile_pool(name="px", bufs=2, space="PSUM"))
        pppool = ctx.enter_context(tc.tile_pool(name="pp", bufs=1, space="PSUM"))

        # small tiles first on the sync queue; big as_s pieces last
        ad_s = cpool.tile([128, NW * 4], F16)
        nc.sync.dma_start(ad_s[:], adin[:])
        msh_s = cpool.tile([128, 1], F32)
        nc.sync.dma_start(msh_s[:], mshift[:])
        id16 = cpool.tile([128, 128], F16)
        nc.sync.dma_start(id16[:], ident16[:])
        alslope = cpool.tile([128, 1], F32)
        nc.vector.memset(alslope[:], NEG_SLOPE)
        if not last:
            W_s = cpool.tile([128, 2 * 264], F16)
            nc.sync.dma_start(W_s[:], Wext[:])
            b_s = cpool.tile([1, 264], F16)
            nc.sync.dma_start(b_s[:], bext[:])
            o_s = cpool.tile([1, 128], F16)
            nc.sync.dma_start(o_s[:], ones1[:])
        else:
            fcw_s = cpool.tile([128, 256], F32)
            nc.sync.dma_start(fcw_s[:], fcw[:])
            pp = pppool.tile([128, 256], F32)
        # two separate as tiles: tile-granular dep tracking would otherwise
        # make the first softmax wait for the full 6.9MB table
        aoff = int(offs[asplit]) * 4
        as_t = cpool.tile([128, TOT * 4 - aoff], F16)   # tail words (run first)
        nc.sync.dma_start(as_t[:], asst[:, aoff:])
        if last:
            oh_s = cpool.tile([128, NW * 128], F16)
            nc.sync.dma_start(oh_s[:], onehot[:])
        as_h = cpool.tile([128, aoff], F16)             # head words
        nc.sync.dma_start(as_h[:], asst[:, :aoff])

        def as_view(w, k, o4):
            if w >= asplit:
                return _ap(as_t[:], o4 - aoff, [list(as_t[:].ap[0]), [4, k], [1, 4]])
            return _ap(as_h[:], o4, [list(as_h[:].ap[0]), [4, k], [1, 4]])

        # lg -> (prelu) lr -> (exp) back into lg: lg doubles as e_all
        lg = cpool.tile([128, TOT * 4], F16)
        lr = cpool.tile([128, TOT * 4], F16)
        e_all = lg
        etw = cpool.tile([128, TOT * 8], F16)
        den = cpool.tile([128, NW * 4], F32)
        rde = cpool.tile([128, NW * 4], F32)

        def softmax_chunk(ci):
            c0, c1 = chunks[ci]
            o4c, o4e = int(offs[c0]) * 4, int(offs[c1]) * 4
            for w in range(c0, c1):
                k = int(kT[w])
                o4 = int(offs[w]) * 4
                ad_b = _ap(ad_s[:], w * 4, [list(ad_s[:].ap[0]), [0, k], [1, 4]])
                as_v = as_view(w, k, o4)
                nc.vector.tensor_tensor(
                    out=_ap(lg[:], o4, [list(lg[:].ap[0]), [4, k], [1, 4]]),
                    in0=as_v, in1=ad_b, op=mybir.AluOpType.add)
            nc.scalar.activation(lr[:, o4c:o4e], lg[:, o4c:o4e],
                                 mybir.ActivationFunctionType.Prelu,
                                 alpha=alslope[:])
            nc.scalar.activation(e_all[:, o4c:o4e], lr[:, o4c:o4e],
                                 mybir.ActivationFunctionType.Exp,
                                 bias=msh_s[:], scale=1.0)
            for w in range(c0, c1):
                k = int(kT[w])
                o4 = int(offs[w]) * 4
                nc.vector.reduce_sum(
                    den[:, w * 4:(w + 1) * 4],
                    _ap(e_all[:], o4, [list(e_all[:].ap[0]), [1, 4], [4, k]]),
                    axis=mybir.AxisListType.X)
            nc.vector.reciprocal(rde[:, c0 * 4:c1 * 4], den[:, c0 * 4:c1 * 4])
            for w in range(c0, c1):
                k = int(kT[w])
                o4 = int(offs[w]) * 4
                nc.vector.tensor_tensor(
                    out=_ap(etw[:], int(offs[w]) * 8,
                            [list(etw[:].ap[0]), [8, k], [2, 4], [1, 2]]),
                    in0=_ap(e_all[:], o4,
                            [list(e_all[:].ap[0]), [4, k], [1, 4], [0, 2]]),
                    in1=_ap(rde[:], w * 4,
                            [list(rde[:].ap[0]), [0, k], [1, 4], [0, 2]]),
                    op=mybir.AluOpType.mult)

        gtiles = {}

        def issue_chunk(ci):
            c0, c1 = chunks[ci]
            S = int(offs[c1] - offs[c0])
            g = gpool.tile([128, SMAX * 256], F16, tag="g")
            nc.gpsimd.dma_start(
                g[:, :S * 256],
                stream[:, int(offs[c0]) * 256:int(offs[c1]) * 256])
            gtiles[ci] = g

        # process chunks in reverse (last chunks have many small words;
        # ending on a few-word chunk shortens the epilogue tail)
        order = list(range(NCH))[::-1]
        issue_chunk(order[0])
        if NCH > 1:
            issue_chunk(order[1])
        softmax_chunk(order[0])

        pend = None      # deferred epilogue: (kind, args)
        hos_pend = None  # (hos_tile, c0, c1) awaiting last-word epilogue + DMA

        def emit_epilogue(pw):
            w, ofs_t, ob, hos_t, hb = pw
            outT = spool.tile([128, 256], F16, tag="outT")
            pt = ptpool.tile([128, 256], F16, tag="pt")
            for q in range(2):
                nc.tensor.matmul(pt[:, q * 128:(q + 1) * 128],
                                 lhsT=ofs_t[:, ob + q * 128:ob + (q + 1) * 128],
                                 rhs=id16[:], is_transpose=True,
                                 start=True, stop=True,
                                 skip_group_check=True)
            nc.scalar.copy(outT[:], pt[:])
            hp = pxpool.tile([128, 264], F32, tag="hp")
            for q in range(2):
                nc.tensor.matmul(hp[:], lhsT=outT[:, q * 128:(q + 1) * 128],
                                 rhs=W_s[:, q * 264:(q + 1) * 264],
                                 start=(q == 0), stop=False)
            nc.tensor.matmul(hp[:], lhsT=o_s[:], rhs=b_s[:],
                             start=False, stop=True)
            nc.scalar.copy(hos_t[:, hb * 264:(hb + 1) * 264], hp[:])

        for oi, ci in enumerate(order):
            c0, c1 = chunks[ci]
            if oi + 2 < NCH:
                issue_chunk(order[oi + 2])
            if oi + 1 < NCH:
                softmax_chunk(order[oi + 1])
            g = gtiles.pop(ci)
            nw_c = c1 - c0
            # msg = h * alpha for the whole chunk (one DVE op, 2x mode)
            S = int(offs[c1] - offs[c0])
            ge = _ap(g[:], 0, [list(g[:].ap[0]), [64, S * 4], [2, 32], [1, 2]])
            ee = _ap(etw[:], int(offs[c0]) * 8,
                     [list(etw[:].ap[0]), [2, S * 4], [0, 32], [1, 2]])
            nc.vector.tensor_tensor(out=ge, in0=ge, in1=ee, op=mybir.AluOpType.mult)
            of = ofpool.tile([128, WCH * 512], F16, tag="of")
            if not last:
                hos = hopool.tile([128, WCH * 264], F16, tag="hos")
                ofs = ofpool.tile([128, WCH * 256], F16, tag="ofs")
            for w in range(c0, c1):
                k = int(kT[w])
                gb = (int(offs[w]) - int(offs[c0])) * 256  # elem offset in g
                # slot-sum: FD=512 pairs into psum halves, odd tail into half A
                ps = pspool.tile([128, 512], F32, tag="ps")
                np2 = k // 2
                for t in range(np2):
                    nc.tensor.matmul(
                        ps[:], lhsT=id16[:],
                        rhs=_ap(g[:], gb + 2 * t * 256,
                                [list(g[:].ap[0]), [1, 512]]),
                        start=(t == 0), stop=(t == np2 - 1 and k % 2 == 0))
                if k % 2 == 1:
                    nc.tensor.matmul(
                        ps[:, 0:256], lhsT=id16[:],
                        rhs=_ap(g[:], gb + (k - 1) * 256,
                                [list(g[:].ap[0]), [1, 256]]),
                        start=False, stop=True, skip_group_check=True)
                ob = (w - c0) * 512
                nc.scalar.copy(of[:, ob:ob + 512], ps[:])
                if not last:
                    sb = (w - c0) * 256
                    nc.vector.tensor_tensor(
                        out=ofs[:, sb:sb + 256], in0=of[:, ob:ob + 256],
                        in1=of[:, ob + 256:ob + 512], op=mybir.AluOpType.add)
                    if pend is not None:
                        emit_epilogue(pend)
                    pend = (w, ofs, sb, hos, w - c0)
                else:
                    nc.tensor.matmul(pp[:], lhsT=oh_s[:, w * 128:(w + 1) * 128],
                                     rhs=of[:, ob:ob + 256],
                                     start=(oi == 0 and w == c0), stop=False,
                                     skip_group_check=True)
                    last_word = (oi == NCH - 1 and w == c1 - 1)
                    nc.tensor.matmul(pp[:], lhsT=oh_s[:, w * 128:(w + 1) * 128],
                                     rhs=of[:, ob + 256:ob + 512],
                                     start=False, stop=last_word,
                                     skip_group_check=True)
            if not last:
                if hos_pend is not None:
                    # flush previous chunk: its last word is in pend? no --
                    # pend now belongs to this chunk; previous chunk fully
                    # epilogued except its last word, which was emitted as the
                    # first pend flush above. DMA it out now.
                    ph, pc0, pc1 = hos_pend
                    nc.sync.dma_start(hout[:, pc0 * 264:pc1 * 264],
                                      ph[:, :(pc1 - pc0) * 264])
                hos_pend = (hos, c0, c1)
        if not last:
            if pend is not None:
                emit_epilogue(pend)
            if hos_pend is not None:
                ph, pc0, pc1 = hos_pend
                nc.sync.dma_start(hout[:, pc0 * 264:pc1 * 264],
                                  ph[:, :(pc1 - pc0) * 264])
        if last:
            fm = spool.tile([128, 256], F32, tag="fm")
            nc.vector.tensor_tensor(out=fm[:], in0=pp[:], in1=fcw_s[:],
                                    op=mybir.AluOpType.mult)
            pv = spool.tile([128, 1], F32, tag="pv")
            nc.vector.reduce_sum(pv[:], fm[:], axis=mybir.AxisListType.X)
            nc.sync.dma_start(pout[:], pv[:])
    nc.compile()
    return nc


# ---------------------------------------------------------------- run

def _run(nc, in_maps):
    trace = _trace_on() and _install_profhook()
    res = bass_utils.run_bass_kernel_spmd(
        nc, in_maps=in_maps, core_ids=list(range(NCORES)), trace=trace
    )
    if _trace_on():
        _EXEC_NS.append(res.exec_time_ns)
    return res


def kernel(x, edge_index, batch, W1, a_src1, a_dst1, b1, W2, a_src2, a_dst2, b2,
           W3, a_src3, a_dst3, b3, fc_W, fc_b):
    _EXEC_NS.clear()
    x = np.asarray(x, np.float32)
    edge_index = np.asarray(edge_index)
    batch = np.asarray(batch)
    meta = build_meta(edge_index)
    build_pool(meta, batch)
    NW, TOT = meta["NW"], meta["TOT"]
    id16 = np.eye(128, dtype=np.float16)
    ones1 = np.ones((1, 128), np.float16)

    wext1 = make_wext(W1, a_src1, a_dst1).astype(np.float16)
    bext1 = make_bext(b1, a_src1, a_dst1).astype(np.float16).reshape(1, 264)
    nc0 = build_l0(meta)
    in0 = []
    for cd in meta["cores"]:
        xp = np.zeros((NW * 128, 128), np.float16)
        real = cd["perm"] >= 0
        xp[real] = x[cd["perm"][real]].astype(np.float16)
        in0.append({"xT": np.ascontiguousarray(xp.T), "Wext": wext1, "bext": bext1,
                    "ones1": ones1})
    r0 = _run(nc0, in0)
    # hout layout [128, NW*264] -> per-node rows [NW*128, 264]
    houts = [r0.results[c]["hout"].reshape(128, NW, 264)
             .transpose(1, 0, 2).reshape(NW * 128, 264)
             for c in range(NCORES)]

    nc_mid = build_agg(meta, last=False)
    nc_last = build_agg(meta, last=True)

    wexts = [make_wext(W2, a_src2, a_dst2).astype(np.float16),
             make_wext(W3, a_src3, a_dst3).astype(np.float16), None]
    bexts = [make_bext(b2, a_src2, a_dst2).astype(np.float16).reshape(1, 264),
             make_bext(b3, a_src3, a_dst3).astype(np.float16).reshape(1, 264), None]

    for li in range(3):
        last = li == 2
        # assemble global tables from per-core houts
        tab_h = np.zeros((TROWS, 256), np.float32)
        tab_as = np.zeros((TROWS, 4), np.float16)
        for cd, h in zip(meta["cores"], houts):
            real = cd["perm"] >= 0
            tab_h[cd["perm"][real]] = h[real, 0:256].astype(np.float32)
            tab_as[cd["perm"][real]] = h[real, 256:260]
        tab_as[ZROW] = np.float16(LOGIT_M[li])
        tab_as[PROW] = np.float16(-30000.0)
        tab8 = np.clip(tab_h, -240, 240).astype(ml_dtypes.float8_e4m3fn)

        ims = []
        for c, cd in enumerate(meta["cores"]):
            stream = tab8[cd["idx"]].reshape(128, TOT * 256)
            asst = tab_as[cd["idx"]].reshape(128, TOT * 4)
            adin = np.ascontiguousarray(
                houts[c].reshape(NW, 128, 264)[:, :, 260:264]
                .transpose(1, 0, 2).reshape(128, NW * 4)).astype(np.float16)
            im = {"stream": stream, "asst": asst, "adin": adin,
                  "mshift": np.full((128, 1), -LOGIT_M[li], np.float32),
                  "ident16": id16}
            if not last:
                W2c = wexts[li]  # [256, 264]
                im["Wext"] = np.ascontiguousarray(
                    np.concatenate([W2c[0:128, :], W2c[128:256, :]], axis=1))
                im["bext"] = bexts[li]
                im["ones1"] = ones1
            else:
                im["onehot"] = cd["pool_onehot"]
                im["fcw"] = np.tile(np.asarray(fc_W, np.float32).reshape(1, 256), (128, 1))
            ims.append(im)
        rr = _run(nc_mid if not last else nc_last, ims)
        if not last:
            houts = [rr.results[c]["hout"].reshape(128, NW, 264)
                     .transpose(1, 0, 2).reshape(NW * 128, 264)
                     for c in range(NCORES)]
        else:
            outv = np.zeros(N_GRAPHS, np.float64)
            for c, cd in enumerate(meta["cores"]):
                pv = rr.results[c]["pout"].reshape(128)
                gb = cd["gbase"]
                hi = min(128, N_GRAPHS - gb)
                outv[gb:gb + hi] += pv[:hi]
            bias_fc = float(np.asarray(b3, np.float64) @ np.asarray(fc_W, np.float64).reshape(-1))
            outv += meta["gcounts"] * bias_fc
            outv += float(np.asarray(fc_b, np.float64).reshape(()))
    return outv.reshape(N_GRAPHS, 1).astype(np.float32)
